# revision 25
# baseline (speedup 1.0000x reference)
"""KNN block-edge kernel for Trainium2 (8 NeuronCores, one segment per core).

Problem (hardcoded from the reference):
  B=8 segments x NPER=512 blocks x U=4 units, 3-D positions, K=16.
  Candidate edges = all intra-segment block pairs (row-major, C=512 per row).
  Block-block distance = min over the 4x4 unit pairs of Euclidean distance.
  Output = per row the K nearest candidate edges, distance-ascending
  (ties: ascending edge index), as (row_o, col_o, attr) int32 arrays.

Device strategy per core (segment b):
  PE computes s(iu, jv) = 2*x.y - |x|^2 - |y|^2 = -d2 for unit pairs via
  K=30 bf16 matmuls: each fp32 operand column is split into three bf16
  terms (hi/mid/lo) and six cross-term groups accumulate in fp32 PSUM,
  giving ~2^-24 relative accuracy at bf16 streaming speed.  The four
  lhs-unit variants run CONCURRENTLY in 4x row-group tiling (32x128 tiles,
  K=30<=32), each writing its own PSUM bank.
  The 512x512 block score matrix is symmetric: only the 10 upper-triangle
  128x128-block chunks are computed; the 6 lower-triangle blocks are PE
  transposes of finished upper blocks (evacuated by ScalarE).
  Per chunk, the 16:1 (u,v) max-pool runs as ONE VectorE tensor_reduce
  (axis=XY) straight out of PSUM.
  Top-16 per row: each 128-column quarter yields its top-8 values+indices
  (max8 + find_index8, exact fp32).  The host merges 4x8 candidates per
  row; rows where one quarter may hide more than 8 of the true top-16
  (~10%) are recomputed exactly on the host.
"""

import numpy as np

B = 8
NPER = 512
U = 4
KTOP = 16
NU = NPER * U          # units per segment (2048)
NBLK = B * NPER        # total blocks (4096)
MT = NPER // 128       # row tiles per core (4)
KC = 30                # contraction: 6 groups x 5 (3-term bf16 split)
NQ = 4                 # quarters per row
LCOLS = NPER           # lhsT columns per row-group (4 t-tiles x 128)
RCOLS = NU             # rhs columns per row-group (all units)

_cache = {}


def _build_bass():
    import concourse.bacc as bacc
    import concourse.mybir as mybir
    from concourse import masks
    from concourse.tile import TileContext

    f32 = mybir.dt.float32
    bf16 = mybir.dt.bfloat16
    u32 = mybir.dt.uint32
    Alu = mybir.AluOpType

    nc = bacc.Bacc("TRN2")
    # Row-group layout: partition rows [32u, 32u+30) hold u's operands:
    #   cols [0, 512): lhsT columns, col = t*128 + i  (block t*128+i, unit u)
    #   cols [512, 2560): rhs columns, col = 512 + unit index (j*4 + v)
    ops = nc.dram_tensor("ops", [128, LCOLS + RCOLS], bf16, kind="ExternalInput")
    out_val = nc.dram_tensor("out_val", [MT, 128, 32], f32, kind="ExternalOutput")
    out_idx = nc.dram_tensor("out_idx", [MT, 128, 32], u32, kind="ExternalOutput")

    with TileContext(nc) as tc:
        with (
            tc.tile_pool(name="const", bufs=1) as cpool,
            tc.tile_pool(name="psum", bufs=1, space="PSUM") as ppool,
            tc.tile_pool(name="work", bufs=1) as wpool,
        ):
            # split the input DMA so chunk (0,0) can start before the rest
            # of the rhs lands; issue from the vector/scalar queues, whose
            # preamble finishes ~2us before sync's
            ops_sb = cpool.tile([128, LCOLS + RCOLS], bf16)
            head = LCOLS + NPER
            nc.sync.dma_start(out=ops_sb[:, :head], in_=ops[:, :head])
            nc.sync.dma_start(out=ops_sb[:, head:], in_=ops[:, head:])

            identity = cpool.tile([128, 128], f32)
            masks.make_identity(nc, identity[:])

            # persistent per-row-tile score tiles  s = -d2min  [128, 512]
            s_tiles = [cpool.tile([128, NPER], f32, name=f"s{t}") for t in range(MT)]

            # warm the ACT table set (Copy) off the critical path
            warm = cpool.tile([128, 8], f32)
            nc.scalar.copy(warm, identity[:, :8])

            def emit_chunk(t, c):
                """PE: 4 concurrent row-group matmuls into one 4-bank PSUM
                chunk; DVE pools 16:1 into s_tiles[t][:, c*128:(c+1)*128]."""
                ps = ppool.tile([128, NU], f32, tag="chunk", bufs=2, name="ps")
                for u in range(U):
                    nc.tensor.matmul(
                        ps[:, u * NPER:(u + 1) * NPER],
                        lhsT=ops_sb[32 * u:32 * u + KC, t * 128:(t + 1) * 128],
                        rhs=ops_sb[32 * u:32 * u + KC,
                                   LCOLS + c * NPER:LCOLS + (c + 1) * NPER],
                        start=True,
                        stop=True,
                        tile_position=(32 * u, 0),
                    )
                dst = s_tiles[t][:, c * 128:(c + 1) * 128]
                ps4 = ps.rearrange("p (u j v) -> p j u v", u=U, v=U)
                nc.vector.tensor_reduce(dst, ps4, mybir.AxisListType.XY, Alu.max)

            def emit_transposes(pairs):
                """pairs: list of (t, c) with c < t; s_t[:, c] = s_c[:, t].T"""
                tp = ppool.tile([128, NU], f32, tag="chunk", bufs=2, name="tp")
                for k, (t, c) in enumerate(pairs):
                    src = s_tiles[c][:, t * 128:(t + 1) * 128]
                    pslice = tp[:, k * NPER:k * NPER + 128]
                    nc.tensor.transpose(pslice, src, identity[:])
                    nc.scalar.copy(s_tiles[t][:, c * 128:(c + 1) * 128], pslice)

            vqs, iqs = {}, {}

            def emit_topk(t, quarters, dma=False, eng=None):
                if t not in vqs:
                    vqs[t] = wpool.tile([128, 32], f32, tag="vq", bufs=2,
                                        name=f"vq{t}")
                    iqs[t] = wpool.tile([128, 32], u32, tag="iq", bufs=2,
                                        name=f"iq{t}")
                vq, iq = vqs[t], iqs[t]
                for q in quarters:
                    sq = s_tiles[t][:, q * 128:(q + 1) * 128]
                    nc.vector.max(out=vq[:, q * 8:(q + 1) * 8], in_=sq)
                    nc.vector.max_index(
                        out=iq[:, q * 8:(q + 1) * 8],
                        in_max=vq[:, q * 8:(q + 1) * 8],
                        in_values=sq,
                    )
                if dma:
                    eng = eng or nc.sync
                    eng.dma_start(out=out_val[t], in_=vq)
                    eng.dma_start(out=out_idx[t], in_=iq)

            # Upper-triangle chunks; mirror via PE transpose.  topk for row t
            # is emitted a row late so DVE never stalls on it; the diagonal
            # chunk (3,3) is hoisted before (2,3) so only a short transpose +
            # two small DVE ops remain after the final reduce.
            emit_chunk(0, 0); emit_chunk(0, 1); emit_chunk(0, 2); emit_chunk(0, 3)
            emit_transposes([(1, 0), (2, 0), (3, 0)])
            emit_chunk(1, 1); emit_chunk(1, 2); emit_chunk(1, 3)
            emit_transposes([(2, 1), (3, 1)])
            emit_topk(0, range(NQ), dma=True)
            emit_chunk(2, 2)
            emit_chunk(3, 3)
            emit_topk(1, range(NQ), dma=True)
            # quarters whose inputs are ready before the final reduce
            emit_topk(3, [0, 1, 3])
            emit_topk(2, [0, 1, 2])
            emit_chunk(2, 3)
            emit_transposes([(3, 2)])
            emit_topk(2, [3], dma=True, eng=nc.scalar)
            emit_topk(3, [2], dma=True, eng=nc.scalar)
    nc.compile()
    return nc


def _get_nc():
    if "nc" not in _cache:
        _cache["nc"] = _build_bass()
    return _cache["nc"]


def _bf16(x):
    from ml_dtypes import bfloat16
    return x.astype(bfloat16).astype(np.float32)


def _split3(x):
    h = _bf16(x)
    m = _bf16(x - h)
    l = _bf16(x - h - m)
    return h, m, l


def _make_core_inputs(unit_pos):
    """Per-core operand tensor [128, 2560] bf16 (row-group layout).

    fp32 augmented columns: A = [2x, -|x|^2, -1] (lhs), B = [y, 1, |y|^2]
    (rhs) so A.B = -d2.  Each is split into three bf16 terms (h/m/l); six
    cross-term groups stack along K (30 rows):
      lhsT rows: [Ah; Am; Al; Ah; Am; Ah]   rhs rows: [Bh; Bh; Bh; Bm; Bm; Bl]
    -> sum = AhBh+AmBh+AlBh+AhBm+AmBm+AhBl ~= A.B to ~3e-8 relative.
    Row-group u (partitions 32u..32u+29) holds unit-u lhs columns and a
    full copy of the rhs.
    """
    from ml_dtypes import bfloat16

    in_maps = []
    for b in range(B):
        P = np.ascontiguousarray(unit_pos[b * NU:(b + 1) * NU]).astype(
            np.float32, copy=False)
        n = (P * P).sum(axis=1, dtype=np.float32)
        A = np.concatenate(
            [2.0 * P, -n[:, None], -np.ones((NU, 1), np.float32)], axis=1)
        Bm = np.concatenate(
            [P, np.ones((NU, 1), np.float32), n[:, None]], axis=1)
        Ah, Am, Al = _split3(A)
        Bh, Bmid, Bl = _split3(Bm)
        lhs = np.concatenate([Ah, Am, Al, Ah, Am, Ah], axis=1).T  # [30, 2048]
        rhs = np.concatenate([Bh, Bh, Bh, Bmid, Bmid, Bl], axis=1).T
        ops = np.zeros((128, LCOLS + RCOLS), np.float32)
        for u in range(U):
            # lhs cols for unit u: block index = t*128+i = 0..511 in order
            ops[32 * u:32 * u + KC, :LCOLS] = lhs[:, u::U]
            ops[32 * u:32 * u + KC, LCOLS:] = rhs
        in_maps.append({"ops": ops.astype(bfloat16)})
    return in_maps


def _run_device(in_maps, trace=False):
    from concourse.bass_utils import run_bass_kernel_spmd

    nc = _get_nc()
    return run_bass_kernel_spmd(nc, in_maps, core_ids=list(range(B)), trace=trace)


def _ref_row_topk(P, n, i_local):
    """Reference-exact (fp32) top-16 local column indices for one row."""
    Pi = P[i_local * U:(i_local + 1) * U]                     # [4, 3]
    ni = n[i_local * U:(i_local + 1) * U]
    d2 = ni[:, None] + n[None, :] - 2.0 * (Pi @ P.T).astype(np.float32)
    dist = np.sqrt(np.maximum(d2, 0.0)).reshape(U, NPER, U).min(axis=(0, 2))
    return np.argsort(dist, kind="stable")[:KTOP]


def _postprocess(results, row, col, unit_pos):
    row_mat = row.reshape(NBLK, NPER)
    col_mat = col.reshape(NBLK, NPER)
    row_o = np.empty((NBLK, KTOP), np.int32)
    col_o = np.empty((NBLK, KTOP), np.int32)
    unit_pos = np.asarray(unit_pos, np.float32)
    qoff = np.repeat(np.arange(NQ, dtype=np.int64) * 128, 8)[None, :]
    ridx = np.arange(NPER)[:, None]
    for b in range(B):
        vals = results[b]["out_val"].reshape(NPER, 32)     # s = -d2, 4 quarters x8
        idxs = results[b]["out_idx"].reshape(NPER, 32).astype(np.int64) + qoff
        # merge: sort 32 candidates by (d2 asc, idx asc) — reference tie
        # semantics.  Pre-sort by idx, then stable-sort by d2.
        d2c = -vals
        pre = np.argsort(idxs, axis=1, kind="stable")
        d2s = d2c[ridx, pre]
        idxs_s = idxs[ridx, pre]
        ordv = np.argsort(d2s, axis=1, kind="stable")[:, :KTOP]
        top_idx = idxs_s[ridx, ordv]
        top_d2 = d2s[ridx, ordv]

        # flag rows for exact recompute:
        #  (a) a quarter contributed all 8 of its candidates to the top-16
        #      (its unreturned 9th might also belong)
        #  (b) duplicate indices (find_index8 value tie within a quarter)
        #  (c) integrity: quarter values must be descending, indices in
        #      [0,128), self edge (d2~0) at rank 1 — catches any corruption
        qsrc = top_idx // 128
        cnt = np.stack([(qsrc == q).sum(1) for q in range(NQ)], 1)
        bad = (cnt >= 8).any(1)
        si = np.sort(top_idx, axis=1)
        bad |= (np.diff(si, axis=1) == 0).any(1)
        v4 = vals.reshape(NPER, NQ, 8)
        bad |= (np.diff(v4, axis=2) > 0).any(axis=(1, 2))
        bad |= (results[b]["out_idx"].reshape(NPER, 32) >= 128).any(1)
        bad |= top_idx[:, 0] != np.arange(NPER)
        bad |= np.abs(top_d2[:, 0]) > 1e-2
        if bad.any():
            P = unit_pos[b * NU:(b + 1) * NU]
            n = (P * P).sum(axis=1, dtype=np.float32)
            for rloc in np.flatnonzero(bad):
                top_idx[rloc] = _ref_row_topk(P, n, rloc)
        gr = slice(b * NPER, (b + 1) * NPER)
        row_o[gr] = row_mat[gr][ridx, top_idx]
        col_o[gr] = col_mat[gr][ridx, top_idx]
    attr = np.zeros(NBLK * KTOP, np.int32)
    return row_o.reshape(-1), col_o.reshape(-1), attr


def kernel(unit_pos, row, col, unit2block, segment_ids, k):
    unit_pos = np.asarray(unit_pos, dtype=np.float32)
    row = np.asarray(row, dtype=np.int32)
    col = np.asarray(col, dtype=np.int32)
    assert int(k) == KTOP
    in_maps = _make_core_inputs(unit_pos)
    res = _run_device(in_maps, trace=False)
    return _postprocess(res.results, row, col, unit_pos)


# revision 26
# speedup vs baseline: 1.0184x; 1.0184x over previous
"""KNN block-edge kernel for Trainium2 (8 NeuronCores, one segment per core).

Problem (hardcoded from the reference):
  B=8 segments x NPER=512 blocks x U=4 units, 3-D positions, K=16.
  Candidate edges = all intra-segment block pairs (row-major, C=512 per row).
  Block-block distance = min over the 4x4 unit pairs of Euclidean distance.
  Output = per row the K nearest candidate edges, distance-ascending
  (ties: ascending edge index), as (row_o, col_o, attr) int32 arrays.

Device strategy per core (segment b):
  PE computes s(iu, jv) = 2*x.y - |x|^2 - |y|^2 = -d2 for unit pairs via
  K=30 bf16 matmuls: each fp32 operand column is split into three bf16
  terms (hi/mid/lo) and six cross-term groups accumulate in fp32 PSUM,
  giving ~2^-24 relative accuracy at bf16 streaming speed.  The four
  lhs-unit variants run CONCURRENTLY in 4x row-group tiling (32x128 tiles,
  K=30<=32), each writing its own PSUM bank.
  The 512x512 block score matrix is symmetric: only the 10 upper-triangle
  128x128-block chunks are computed; the 6 lower-triangle blocks are PE
  transposes of finished upper blocks (evacuated by ScalarE).
  Per chunk, the 16:1 (u,v) max-pool runs as ONE VectorE tensor_reduce
  (axis=XY) straight out of PSUM.
  Top-16 per row: each 128-column quarter yields its top-8 values+indices
  (max8 + find_index8, exact fp32).  The host merges 4x8 candidates per
  row; rows where one quarter may hide more than 8 of the true top-16
  (~10%) are recomputed exactly on the host.
"""

import numpy as np

B = 8
NPER = 512
U = 4
KTOP = 16
NU = NPER * U          # units per segment (2048)
NBLK = B * NPER        # total blocks (4096)
MT = NPER // 128       # row tiles per core (4)
KC = 30                # contraction: 6 groups x 5 (3-term bf16 split)
NQ = 4                 # quarters per row
LCOLS = NPER           # lhsT columns per row-group (4 t-tiles x 128)
RCOLS = NU             # rhs columns per row-group (all units)

_cache = {}


def _build_bass():
    import concourse.bacc as bacc
    import concourse.mybir as mybir
    from concourse import masks
    from concourse.tile import TileContext

    f32 = mybir.dt.float32
    bf16 = mybir.dt.bfloat16
    u32 = mybir.dt.uint32
    Alu = mybir.AluOpType

    nc = bacc.Bacc("TRN2")
    # Row-group layout: partition rows [32u, 32u+30) hold u's operands:
    #   cols [0, 512): lhsT columns, col = t*128 + i  (block t*128+i, unit u)
    #   cols [512, 2560): rhs columns, col = 512 + unit index (j*4 + v)
    ops = nc.dram_tensor("ops", [128, LCOLS + RCOLS], bf16, kind="ExternalInput")
    out_val = nc.dram_tensor("out_val", [MT, 128, 32], f32, kind="ExternalOutput")
    out_idx = nc.dram_tensor("out_idx", [MT, 128, 32], u32, kind="ExternalOutput")

    with TileContext(nc) as tc:
        with (
            tc.tile_pool(name="const", bufs=1) as cpool,
            tc.tile_pool(name="psum", bufs=1, space="PSUM") as ppool,
            tc.tile_pool(name="work", bufs=1) as wpool,
        ):
            # split the input DMA so chunk (0,0) can start before the rest
            # of the rhs lands; issue from the vector/scalar queues, whose
            # preamble finishes ~2us before sync's
            ops_sb = cpool.tile([128, LCOLS + RCOLS], bf16)
            head = LCOLS + NPER
            nc.sync.dma_start(out=ops_sb[:, :head], in_=ops[:, :head])
            nc.sync.dma_start(out=ops_sb[:, head:], in_=ops[:, head:])

            identity = cpool.tile([128, 128], f32)
            masks.make_identity(nc, identity[:])

            # persistent per-row-tile score tiles  s = -d2min  [128, 512]
            s_tiles = [cpool.tile([128, NPER], f32, name=f"s{t}") for t in range(MT)]

            # warm the ACT table set (Copy) off the critical path
            warm = cpool.tile([128, 8], f32)
            nc.scalar.copy(warm, identity[:, :8])

            def emit_chunk(t, c):
                """PE: 4 concurrent row-group matmuls into one 4-bank PSUM
                chunk; DVE pools 16:1 into s_tiles[t][:, c*128:(c+1)*128]."""
                ps = ppool.tile([128, NU], f32, tag="chunk", bufs=2, name="ps")
                for u in range(U):
                    nc.tensor.matmul(
                        ps[:, u * NPER:(u + 1) * NPER],
                        lhsT=ops_sb[32 * u:32 * u + KC, t * 128:(t + 1) * 128],
                        rhs=ops_sb[32 * u:32 * u + KC,
                                   LCOLS + c * NPER:LCOLS + (c + 1) * NPER],
                        start=True,
                        stop=True,
                        tile_position=(32 * u, 0),
                    )
                dst = s_tiles[t][:, c * 128:(c + 1) * 128]
                ps4 = ps.rearrange("p (u j v) -> p j u v", u=U, v=U)
                nc.vector.tensor_reduce(dst, ps4, mybir.AxisListType.XY, Alu.max)

            def emit_transposes(pairs):
                """pairs: list of (t, c) with c < t; s_t[:, c] = s_c[:, t].T"""
                tp = ppool.tile([128, NU], f32, tag="chunk", bufs=2, name="tp")
                for k, (t, c) in enumerate(pairs):
                    src = s_tiles[c][:, t * 128:(t + 1) * 128]
                    pslice = tp[:, k * NPER:k * NPER + 128]
                    nc.tensor.transpose(pslice, src, identity[:])
                    nc.scalar.copy(s_tiles[t][:, c * 128:(c + 1) * 128], pslice)

            vqs, iqs = {}, {}

            def emit_topk(t, quarters, dma=False, eng=None):
                if t not in vqs:
                    vqs[t] = wpool.tile([128, 32], f32, tag="vq", bufs=2,
                                        name=f"vq{t}")
                    iqs[t] = wpool.tile([128, 32], u32, tag="iq", bufs=2,
                                        name=f"iq{t}")
                vq, iq = vqs[t], iqs[t]
                for q in quarters:
                    sq = s_tiles[t][:, q * 128:(q + 1) * 128]
                    nc.vector.max(out=vq[:, q * 8:(q + 1) * 8], in_=sq)
                    nc.vector.max_index(
                        out=iq[:, q * 8:(q + 1) * 8],
                        in_max=vq[:, q * 8:(q + 1) * 8],
                        in_values=sq,
                    )
                if dma:
                    eng = eng or nc.sync
                    eng.dma_start(out=out_val[t], in_=vq)
                    eng.dma_start(out=out_idx[t], in_=iq)

            # Upper-triangle chunks; mirror via PE transpose.  topk for row t
            # is emitted a row late so DVE never stalls on it; the diagonal
            # chunk (3,3) is hoisted before (2,3) so only a short transpose +
            # two small DVE ops remain after the final reduce.
            emit_chunk(0, 0); emit_chunk(0, 1); emit_chunk(0, 2); emit_chunk(0, 3)
            emit_transposes([(1, 0), (2, 0), (3, 0)])
            emit_chunk(1, 1); emit_chunk(1, 2); emit_chunk(1, 3)
            emit_transposes([(2, 1), (3, 1)])
            emit_topk(0, range(NQ), dma=True)
            emit_chunk(2, 2)
            emit_chunk(3, 3)
            emit_chunk(2, 3)
            emit_transposes([(3, 2)])
            emit_topk(1, range(NQ), dma=True)
            emit_topk(3, [0, 1, 3])
            emit_topk(2, range(NQ), dma=True, eng=nc.scalar)
            emit_topk(3, [2], dma=True, eng=nc.scalar)
    nc.compile()
    return nc


def _get_nc():
    if "nc" not in _cache:
        _cache["nc"] = _build_bass()
    return _cache["nc"]


def _bf16(x):
    from ml_dtypes import bfloat16
    return x.astype(bfloat16).astype(np.float32)


def _split3(x):
    h = _bf16(x)
    m = _bf16(x - h)
    l = _bf16(x - h - m)
    return h, m, l


def _make_core_inputs(unit_pos):
    """Per-core operand tensor [128, 2560] bf16 (row-group layout).

    fp32 augmented columns: A = [2x, -|x|^2, -1] (lhs), B = [y, 1, |y|^2]
    (rhs) so A.B = -d2.  Each is split into three bf16 terms (h/m/l); six
    cross-term groups stack along K (30 rows):
      lhsT rows: [Ah; Am; Al; Ah; Am; Ah]   rhs rows: [Bh; Bh; Bh; Bm; Bm; Bl]
    -> sum = AhBh+AmBh+AlBh+AhBm+AmBm+AhBl ~= A.B to ~3e-8 relative.
    Row-group u (partitions 32u..32u+29) holds unit-u lhs columns and a
    full copy of the rhs.
    """
    from ml_dtypes import bfloat16

    in_maps = []
    for b in range(B):
        P = np.ascontiguousarray(unit_pos[b * NU:(b + 1) * NU]).astype(
            np.float32, copy=False)
        n = (P * P).sum(axis=1, dtype=np.float32)
        A = np.concatenate(
            [2.0 * P, -n[:, None], -np.ones((NU, 1), np.float32)], axis=1)
        Bm = np.concatenate(
            [P, np.ones((NU, 1), np.float32), n[:, None]], axis=1)
        Ah, Am, Al = _split3(A)
        Bh, Bmid, Bl = _split3(Bm)
        lhs = np.concatenate([Ah, Am, Al, Ah, Am, Ah], axis=1).T  # [30, 2048]
        rhs = np.concatenate([Bh, Bh, Bh, Bmid, Bmid, Bl], axis=1).T
        ops = np.zeros((128, LCOLS + RCOLS), np.float32)
        for u in range(U):
            # lhs cols for unit u: block index = t*128+i = 0..511 in order
            ops[32 * u:32 * u + KC, :LCOLS] = lhs[:, u::U]
            ops[32 * u:32 * u + KC, LCOLS:] = rhs
        in_maps.append({"ops": ops.astype(bfloat16)})
    return in_maps


def _run_device(in_maps, trace=False):
    from concourse.bass_utils import run_bass_kernel_spmd

    nc = _get_nc()
    return run_bass_kernel_spmd(nc, in_maps, core_ids=list(range(B)), trace=trace)


def _ref_row_topk(P, n, i_local):
    """Reference-exact (fp32) top-16 local column indices for one row."""
    Pi = P[i_local * U:(i_local + 1) * U]                     # [4, 3]
    ni = n[i_local * U:(i_local + 1) * U]
    d2 = ni[:, None] + n[None, :] - 2.0 * (Pi @ P.T).astype(np.float32)
    dist = np.sqrt(np.maximum(d2, 0.0)).reshape(U, NPER, U).min(axis=(0, 2))
    return np.argsort(dist, kind="stable")[:KTOP]


def _postprocess(results, row, col, unit_pos):
    row_mat = row.reshape(NBLK, NPER)
    col_mat = col.reshape(NBLK, NPER)
    row_o = np.empty((NBLK, KTOP), np.int32)
    col_o = np.empty((NBLK, KTOP), np.int32)
    unit_pos = np.asarray(unit_pos, np.float32)
    qoff = np.repeat(np.arange(NQ, dtype=np.int64) * 128, 8)[None, :]
    ridx = np.arange(NPER)[:, None]
    for b in range(B):
        vals = results[b]["out_val"].reshape(NPER, 32)     # s = -d2, 4 quarters x8
        idxs = results[b]["out_idx"].reshape(NPER, 32).astype(np.int64) + qoff
        # merge: sort 32 candidates by (d2 asc, idx asc) — reference tie
        # semantics.  Pre-sort by idx, then stable-sort by d2.
        d2c = -vals
        pre = np.argsort(idxs, axis=1, kind="stable")
        d2s = d2c[ridx, pre]
        idxs_s = idxs[ridx, pre]
        ordv = np.argsort(d2s, axis=1, kind="stable")[:, :KTOP]
        top_idx = idxs_s[ridx, ordv]
        top_d2 = d2s[ridx, ordv]

        # flag rows for exact recompute:
        #  (a) a quarter contributed all 8 of its candidates to the top-16
        #      (its unreturned 9th might also belong)
        #  (b) duplicate indices (find_index8 value tie within a quarter)
        #  (c) integrity: quarter values must be descending, indices in
        #      [0,128), self edge (d2~0) at rank 1 — catches any corruption
        qsrc = top_idx // 128
        cnt = np.stack([(qsrc == q).sum(1) for q in range(NQ)], 1)
        bad = (cnt >= 8).any(1)
        si = np.sort(top_idx, axis=1)
        bad |= (np.diff(si, axis=1) == 0).any(1)
        v4 = vals.reshape(NPER, NQ, 8)
        bad |= (np.diff(v4, axis=2) > 0).any(axis=(1, 2))
        bad |= (results[b]["out_idx"].reshape(NPER, 32) >= 128).any(1)
        bad |= top_idx[:, 0] != np.arange(NPER)
        bad |= np.abs(top_d2[:, 0]) > 1e-2
        if bad.any():
            P = unit_pos[b * NU:(b + 1) * NU]
            n = (P * P).sum(axis=1, dtype=np.float32)
            for rloc in np.flatnonzero(bad):
                top_idx[rloc] = _ref_row_topk(P, n, rloc)
        gr = slice(b * NPER, (b + 1) * NPER)
        row_o[gr] = row_mat[gr][ridx, top_idx]
        col_o[gr] = col_mat[gr][ridx, top_idx]
    attr = np.zeros(NBLK * KTOP, np.int32)
    return row_o.reshape(-1), col_o.reshape(-1), attr


def kernel(unit_pos, row, col, unit2block, segment_ids, k):
    unit_pos = np.asarray(unit_pos, dtype=np.float32)
    row = np.asarray(row, dtype=np.int32)
    col = np.asarray(col, dtype=np.int32)
    assert int(k) == KTOP
    in_maps = _make_core_inputs(unit_pos)
    res = _run_device(in_maps, trace=False)
    return _postprocess(res.results, row, col, unit_pos)


# revision 28
# speedup vs baseline: 1.0225x; 1.0041x over previous
"""KNN block-edge kernel for Trainium2 (8 NeuronCores, one segment per core).

Problem (hardcoded from the reference):
  B=8 segments x NPER=512 blocks x U=4 units, 3-D positions, K=16.
  Candidate edges = all intra-segment block pairs (row-major, C=512 per row).
  Block-block distance = min over the 4x4 unit pairs of Euclidean distance.
  Output = per row the K nearest candidate edges, distance-ascending
  (ties: ascending edge index), as (row_o, col_o, attr) int32 arrays.

Device strategy per core (segment b):
  PE computes s(iu, jv) = 2*x.y - |x|^2 - |y|^2 = -d2 for unit pairs via
  K=30 bf16 matmuls: each fp32 operand column is split into three bf16
  terms (hi/mid/lo) and six cross-term groups accumulate in fp32 PSUM,
  giving ~2^-24 relative accuracy at bf16 streaming speed.  The four
  lhs-unit variants run CONCURRENTLY in 4x row-group tiling (32x128 tiles,
  K=30<=32), each writing its own PSUM bank.
  The 512x512 block score matrix is symmetric: only the 10 upper-triangle
  128x128-block chunks are computed; the 6 lower-triangle blocks are PE
  transposes of finished upper blocks (evacuated by ScalarE).
  Per chunk, the 16:1 (u,v) max-pool runs as ONE VectorE tensor_reduce
  (axis=XY) straight out of PSUM.
  Top-16 per row: each 128-column quarter yields its top-8 values+indices
  (max8 + find_index8, exact fp32).  The host merges 4x8 candidates per
  row; rows where one quarter may hide more than 8 of the true top-16
  (~10%) are recomputed exactly on the host.
"""

import numpy as np

B = 8
NPER = 512
U = 4
KTOP = 16
NU = NPER * U          # units per segment (2048)
NBLK = B * NPER        # total blocks (4096)
MT = NPER // 128       # row tiles per core (4)
KC = 30                # contraction: 6 groups x 5 (3-term bf16 split)
NQ = 4                 # quarters per row
LCOLS = NPER           # lhsT columns per row-group (4 t-tiles x 128)
RCOLS = NU             # rhs columns per row-group (all units)

_cache = {}


def _build_bass():
    import concourse.bacc as bacc
    import concourse.mybir as mybir
    from concourse import masks
    from concourse.tile import TileContext

    f32 = mybir.dt.float32
    bf16 = mybir.dt.bfloat16
    u32 = mybir.dt.uint32
    Alu = mybir.AluOpType

    nc = bacc.Bacc("TRN2")
    # Row-group layout: partition rows [32u, 32u+30) hold u's operands:
    #   cols [0, 512): lhsT columns, col = t*128 + i  (block t*128+i, unit u)
    #   cols [512, 2560): rhs columns, col = 512 + unit index (j*4 + v)
    ops = nc.dram_tensor("ops", [128, LCOLS + RCOLS], bf16, kind="ExternalInput")
    out_val = nc.dram_tensor("out_val", [MT, 128, 32], f32, kind="ExternalOutput")
    out_idx = nc.dram_tensor("out_idx", [MT, 128, 32], u32, kind="ExternalOutput")

    with TileContext(nc) as tc:
        with (
            tc.tile_pool(name="const", bufs=1) as cpool,
            tc.tile_pool(name="psum", bufs=1, space="PSUM") as ppool,
            tc.tile_pool(name="work", bufs=1) as wpool,
        ):
            # split the input DMA so chunk (0,0) can start before the rest
            # of the rhs lands
            ops_sb = cpool.tile([128, LCOLS + RCOLS], bf16)
            head = LCOLS + NPER
            nc.sync.dma_start(out=ops_sb[:, :head], in_=ops[:, :head])
            nc.sync.dma_start(out=ops_sb[:, head:], in_=ops[:, head:])

            identity = cpool.tile([128, 128], f32)
            masks.make_identity(nc, identity[:])

            # persistent per-row-tile score tiles  s = -d2min  [128, 512]
            s_tiles = [cpool.tile([128, NPER], f32, name=f"s{t}") for t in range(MT)]

            # warm the ACT table set (Copy) off the critical path
            warm = cpool.tile([128, 8], f32)
            nc.scalar.copy(warm, identity[:, :8])

            def emit_chunk(t, c):
                """PE: 4 concurrent row-group matmuls into one 4-bank PSUM
                chunk; DVE pools 16:1 into s_tiles[t][:, c*128:(c+1)*128]."""
                ps = ppool.tile([128, NU], f32, tag="chunk", bufs=2, name="ps")
                for u in range(U):
                    nc.tensor.matmul(
                        ps[:, u * NPER:(u + 1) * NPER],
                        lhsT=ops_sb[32 * u:32 * u + KC, t * 128:(t + 1) * 128],
                        rhs=ops_sb[32 * u:32 * u + KC,
                                   LCOLS + c * NPER:LCOLS + (c + 1) * NPER],
                        start=True,
                        stop=True,
                        tile_position=(32 * u, 0),
                    )
                dst = s_tiles[t][:, c * 128:(c + 1) * 128]
                ps4 = ps.rearrange("p (u j v) -> p j u v", u=U, v=U)
                nc.vector.tensor_reduce(dst, ps4, mybir.AxisListType.XY, Alu.max)

            def emit_transposes(pairs):
                """pairs: list of (t, c) with c < t; s_t[:, c] = s_c[:, t].T"""
                tp = ppool.tile([128, NU], f32, tag="chunk", bufs=2, name="tp")
                for k, (t, c) in enumerate(pairs):
                    src = s_tiles[c][:, t * 128:(t + 1) * 128]
                    pslice = tp[:, k * NPER:k * NPER + 128]
                    nc.tensor.transpose(pslice, src, identity[:])
                    nc.scalar.copy(s_tiles[t][:, c * 128:(c + 1) * 128], pslice)

            vqs, iqs = {}, {}

            def emit_topk(t, quarters, dma=False, eng=None):
                if t not in vqs:
                    vqs[t] = wpool.tile([128, 32], f32, tag="vq", bufs=2,
                                        name=f"vq{t}")
                    iqs[t] = wpool.tile([128, 32], u32, tag="iq", bufs=2,
                                        name=f"iq{t}")
                vq, iq = vqs[t], iqs[t]
                for q in quarters:
                    sq = s_tiles[t][:, q * 128:(q + 1) * 128]
                    nc.vector.max(out=vq[:, q * 8:(q + 1) * 8], in_=sq)
                    nc.vector.max_index(
                        out=iq[:, q * 8:(q + 1) * 8],
                        in_max=vq[:, q * 8:(q + 1) * 8],
                        in_values=sq,
                    )
                if dma:
                    eng = eng or nc.sync
                    eng.dma_start(out=out_val[t], in_=vq)
                    eng.dma_start(out=out_idx[t], in_=iq)

            # Upper-triangle chunks; mirror via PE transpose.  topk for row t
            # is emitted a row late so DVE never stalls on it; the diagonal
            # chunk (3,3) is hoisted before (2,3) so only a short transpose +
            # two small DVE ops remain after the final reduce.
            emit_chunk(0, 0); emit_chunk(0, 1); emit_chunk(0, 2); emit_chunk(0, 3)
            emit_transposes([(1, 0), (2, 0), (3, 0)])
            emit_chunk(1, 1); emit_chunk(1, 2); emit_chunk(1, 3)
            emit_transposes([(2, 1), (3, 1)])
            emit_topk(0, range(NQ), dma=True)
            emit_chunk(2, 2)
            emit_chunk(3, 3)
            emit_chunk(2, 3)
            emit_transposes([(3, 2)])
            emit_topk(1, range(NQ), dma=True)
            emit_topk(3, [0, 1, 3])
            emit_topk(2, range(NQ), dma=True, eng=nc.scalar)
            emit_topk(3, [2], dma=True, eng=nc.scalar)
    nc.compile()
    return nc


def _get_nc():
    if "nc" not in _cache:
        _cache["nc"] = _build_bass()
    return _cache["nc"]


def _bf16(x):
    from ml_dtypes import bfloat16
    return x.astype(bfloat16).astype(np.float32)


def _split3(x):
    h = _bf16(x)
    m = _bf16(x - h)
    l = _bf16(x - h - m)
    return h, m, l


def _make_core_inputs(unit_pos):
    """Per-core operand tensor [128, 2560] bf16 (row-group layout).

    fp32 augmented columns: A = [2x, -|x|^2, -1] (lhs), B = [y, 1, |y|^2]
    (rhs) so A.B = -d2.  Each is split into three bf16 terms (h/m/l); six
    cross-term groups stack along K (30 rows):
      lhsT rows: [Ah; Am; Al; Ah; Am; Ah]   rhs rows: [Bh; Bh; Bh; Bm; Bm; Bl]
    -> sum = AhBh+AmBh+AlBh+AhBm+AmBm+AhBl ~= A.B to ~3e-8 relative.
    Row-group u (partitions 32u..32u+29) holds unit-u lhs columns and a
    full copy of the rhs.
    """
    from ml_dtypes import bfloat16

    in_maps = []
    for b in range(B):
        P = np.ascontiguousarray(unit_pos[b * NU:(b + 1) * NU]).astype(
            np.float32, copy=False)
        n = (P * P).sum(axis=1, dtype=np.float32)
        A = np.concatenate(
            [2.0 * P, -n[:, None], -np.ones((NU, 1), np.float32)], axis=1)
        Bm = np.concatenate(
            [P, np.ones((NU, 1), np.float32), n[:, None]], axis=1)
        Ah, Am, Al = _split3(A)
        Bh, Bmid, Bl = _split3(Bm)
        lhs = np.concatenate([Ah, Am, Al, Ah, Am, Ah], axis=1).T  # [30, 2048]
        rhs = np.concatenate([Bh, Bh, Bh, Bmid, Bmid, Bl], axis=1).T
        ops = np.zeros((128, LCOLS + RCOLS), np.float32)
        for u in range(U):
            # lhs cols for unit u: block index = t*128+i = 0..511 in order
            ops[32 * u:32 * u + KC, :LCOLS] = lhs[:, u::U]
            ops[32 * u:32 * u + KC, LCOLS:] = rhs
        in_maps.append({"ops": ops.astype(bfloat16)})
    return in_maps


def _run_device(in_maps, trace=False):
    from concourse.bass_utils import run_bass_kernel_spmd

    nc = _get_nc()
    return run_bass_kernel_spmd(nc, in_maps, core_ids=list(range(B)), trace=trace)


def _ref_row_topk(P, n, i_local):
    """Reference-exact (fp32) top-16 local column indices for one row."""
    Pi = P[i_local * U:(i_local + 1) * U]                     # [4, 3]
    ni = n[i_local * U:(i_local + 1) * U]
    d2 = ni[:, None] + n[None, :] - 2.0 * (Pi @ P.T).astype(np.float32)
    dist = np.sqrt(np.maximum(d2, 0.0)).reshape(U, NPER, U).min(axis=(0, 2))
    return np.argsort(dist, kind="stable")[:KTOP]


def _postprocess(results, row, col, unit_pos):
    row_mat = row.reshape(NBLK, NPER)
    col_mat = col.reshape(NBLK, NPER)
    row_o = np.empty((NBLK, KTOP), np.int32)
    col_o = np.empty((NBLK, KTOP), np.int32)
    unit_pos = np.asarray(unit_pos, np.float32)
    qoff = np.repeat(np.arange(NQ, dtype=np.int64) * 128, 8)[None, :]
    ridx = np.arange(NPER)[:, None]
    for b in range(B):
        vals = results[b]["out_val"].reshape(NPER, 32)     # s = -d2, 4 quarters x8
        idxs = results[b]["out_idx"].reshape(NPER, 32).astype(np.int64) + qoff
        # merge: sort 32 candidates by (d2 asc, idx asc) — reference tie
        # semantics.  Pre-sort by idx, then stable-sort by d2.
        d2c = -vals
        pre = np.argsort(idxs, axis=1, kind="stable")
        d2s = d2c[ridx, pre]
        idxs_s = idxs[ridx, pre]
        ordv = np.argsort(d2s, axis=1, kind="stable")[:, :KTOP]
        top_idx = idxs_s[ridx, ordv]
        top_d2 = d2s[ridx, ordv]

        # flag rows for exact recompute:
        #  (a) a quarter contributed all 8 of its candidates to the top-16
        #      (its unreturned 9th might also belong)
        #  (b) duplicate indices (find_index8 value tie within a quarter)
        #  (c) integrity: quarter values must be descending, indices in
        #      [0,128), self edge (d2~0) at rank 1 — catches any corruption
        qsrc = top_idx // 128
        cnt = np.stack([(qsrc == q).sum(1) for q in range(NQ)], 1)
        bad = (cnt >= 8).any(1)
        si = np.sort(top_idx, axis=1)
        bad |= (np.diff(si, axis=1) == 0).any(1)
        v4 = vals.reshape(NPER, NQ, 8)
        bad |= (np.diff(v4, axis=2) > 0).any(axis=(1, 2))
        bad |= (results[b]["out_idx"].reshape(NPER, 32) >= 128).any(1)
        bad |= top_idx[:, 0] != np.arange(NPER)
        bad |= np.abs(top_d2[:, 0]) > 1e-2
        bad |= ~np.isfinite(vals).all(1)
        if bad.any():
            P = unit_pos[b * NU:(b + 1) * NU]
            n = (P * P).sum(axis=1, dtype=np.float32)
            for rloc in np.flatnonzero(bad):
                top_idx[rloc] = _ref_row_topk(P, n, rloc)
        gr = slice(b * NPER, (b + 1) * NPER)
        row_o[gr] = row_mat[gr][ridx, top_idx]
        col_o[gr] = col_mat[gr][ridx, top_idx]
    attr = np.zeros(NBLK * KTOP, np.int32)
    return row_o.reshape(-1), col_o.reshape(-1), attr


def kernel(unit_pos, row, col, unit2block, segment_ids, k):
    unit_pos = np.asarray(unit_pos, dtype=np.float32)
    row = np.asarray(row, dtype=np.int32)
    col = np.asarray(col, dtype=np.int32)
    assert int(k) == KTOP
    in_maps = _make_core_inputs(unit_pos)
    res = _run_device(in_maps, trace=False)
    return _postprocess(res.results, row, col, unit_pos)
